# revision 10
# baseline (speedup 1.0000x reference)
"""Fused quantized Conv2D + BatchNorm block for Trainium2 (8 NeuronCores).

Reference computation (shapes hardcoded):
  x:      [32, 128, 56, 56] f32    activations in [0, 1)
  weight: [256, 128, 3, 3]  f32
  bias/gamma/beta/running_mean/running_var: [256] f32

  xq = round(clip(x,0,4) * 255/4) * (4/255)          (8-bit act quant)
  wq = DoReFa 8-bit weight quant -> values (2k-255)/255, k in 0..255
  y  = conv2d(xq, wq, stride 1, pad 1)               NCHW/OIHW
  out = y * inv + shift      inv = gamma*rsqrt(var+eps), shift = beta - mean*inv + bias

Kernel strategy:
  - Data-parallel over batch: core i handles images [4i, 4i+4).
  - Integer form: a = round(x*63.75) in {0..255}, b = wq*255 odd ints in
    [-255, 255].  Both are exactly representable in bf16, so the conv is
    done as bf16 matmuls with exact fp32 PSUM accumulation; the final
    per-channel scale folds the 4/255^2 factor and BN into one DVE FMA.
  - Conv as implicit GEMM: Cin=128 on the partition (contraction) dim,
    9 tap matmuls accumulate into PSUM.  The quantized input lives
    zero-padded in SBUF (58-wide rows) so each tap is a strided
    [128, 8, 56] view.  Output chunk = 8 rows * 56 cols = 448 <= 512
    (one PSUM bank); the 8 PSUM banks are used round-robin.
  - Each image is split into top (output rows 0..31) / bottom (32..55)
    halves with a 2-row halo so matmuls start after a half-image
    DMA+quantize instead of a full one.
"""

import numpy as np
import ml_dtypes

import concourse.bacc as bacc
import concourse.tile as tile
from concourse import mybir
from concourse.bass_utils import run_bass_kernel_spmd

N_CORES = 8
N_BATCH = 32
IMGS = N_BATCH // N_CORES  # images per core
CIN = 128
COUT = 256
H = W = 56
HW = H * W
WP = 58  # padded row width
KK = 3
NTAPS = KK * KK
RPC = 8  # output rows per chunk
NCHUNKS = H // RPC  # 7
NFREE = RPC * W  # 448
COUT_TILES = COUT // 128  # 2

# top half: padded rows 0..33 (pad row + x rows 0..32), serves chunks 0..3
A_ROWS = 34
A_XROWS = 33  # x rows 0..32
# bottom half: padded rows 32..57 (x rows 31..55 + pad row), serves chunks 4..6
B_ROWS = 26
B_XROWS = 25  # x rows 31..55
B_BASE = 32  # global padded row of local row 0

MAGIC = np.float32(2.0**23)

TRACE = False
TRACE_DIR = None
LAST_RESULT = None  # BassKernelResults of the most recent run (for profiling)

_cached_nc = None


def _build():
    f32 = mybir.dt.float32
    bf16 = mybir.dt.bfloat16
    mult = mybir.AluOpType.mult
    add = mybir.AluOpType.add

    nc = bacc.Bacc("TRN2", target_bir_lowering=False, debug=False,
                   num_devices=N_CORES)
    xs = nc.dram_tensor("xs", [IMGS, CIN, HW], f32, kind="ExternalInput").ap()
    wb = nc.dram_tensor("wb", [CIN, NTAPS * COUT], bf16, kind="ExternalInput").ap()
    sc = nc.dram_tensor("sc", [128, COUT_TILES], f32, kind="ExternalInput").ap()
    sh = nc.dram_tensor("sh", [128, COUT_TILES], f32, kind="ExternalInput").ap()
    ys = nc.dram_tensor("ys", [IMGS, COUT, HW], f32, kind="ExternalOutput").ap()

    with tile.TileContext(nc) as tc:
        with (
            tc.tile_pool(name="wpool", bufs=1) as wpool,
            tc.tile_pool(name="ppool", bufs=1) as ppool,
            tc.tile_pool(name="xpool", bufs=2) as xpool,
            tc.tile_pool(name="apool", bufs=1) as apool,
            tc.tile_pool(name="opool", bufs=16) as opool,
            tc.tile_pool(name="pspool", bufs=1, space="PSUM") as pspool,
        ):
            # Tiny ACT op up front so the activation-table load (~1.3us)
            # overlaps the first input DMA instead of sitting on the
            # quantize critical path.
            warm = ppool.tile([128, 1], f32, tag="warm")
            nc.vector.memset(warm[:], 0.0)
            nc.scalar.activation(warm[:], warm[:],
                                 mybir.ActivationFunctionType.Copy, bias=0.0)

            # Persistent zero-padded activation buffers (2 images in flight).
            # Pad borders are zeroed once; interiors are rewritten per image.
            apadsA, apadsB = [None, None], [None, None]
            for i in range(2):
                ta = apool.tile([CIN, A_ROWS * WP], bf16, tag=f"apadA{i}",
                                name=f"apadA{i}")
                nc.gpsimd.memset(ta[:], 0.0)
                apadsA[i] = ta.rearrange("p (h w) -> p h w", w=WP)
                tb = apool.tile([CIN, B_ROWS * WP], bf16, tag=f"apadB{i}",
                                name=f"apadB{i}")
                nc.gpsimd.memset(tb[:], 0.0)
                apadsB[i] = tb.rearrange("p (h w) -> p h w", w=WP)

            wsb = wpool.tile([CIN, NTAPS * COUT], bf16)
            scb = ppool.tile([128, COUT_TILES], f32, tag="scb")
            shb = ppool.tile([128, COUT_TILES], f32, tag="shb")

            # Per-queue DMA bandwidth caps at ~90 GB/s; only sync/scalar
            # (HWDGE) and gpsimd (SWDGE) can initiate DMAs, so spread
            # traffic across all three queues.
            dma_engs = [nc.sync, nc.scalar, nc.gpsimd]

            def split_rows(lo_row, n_rows, parts):
                base = n_rows // parts
                rows = [base + (1 if i < n_rows % parts else 0)
                        for i in range(parts)]
                out, acc = [], lo_row
                for r in rows:
                    out.append((acc, r))
                    acc += r
                return out

            psum_seq = 0
            out_seq = 0
            for n in range(IMGS):
                a3, b3 = apadsA[n % 2], apadsB[n % 2]
                # top half: x rows 0..32
                xfa = xpool.tile([CIN, A_XROWS * W], f32, tag="xfa",
                                 name=f"xfa{n}")
                if n == 0:
                    # image 0 is on the critical path: split the load across
                    # all three queues
                    for qi, (r0, nr) in enumerate(split_rows(0, A_XROWS, 3)):
                        dma_engs[qi].dma_start(
                            xfa[:, r0 * W: (r0 + nr) * W],
                            xs[n][:, r0 * W: (r0 + nr) * W])
                else:
                    nc.sync.dma_start(xfa[:], xs[n][:, 0:A_XROWS * W])
                nc.vector.tensor_scalar(xfa[:], xfa[:], 63.75, float(MAGIC),
                                        op0=mult, op1=add)
                nc.scalar.activation(
                    a3[:, 1:1 + A_XROWS, 1:1 + W],
                    xfa.rearrange("p (h w) -> p h w", w=W),
                    mybir.ActivationFunctionType.Copy,
                    bias=float(-MAGIC),
                )
                if n == 0:
                    # weights next on the two fast queues (needed by first MM)
                    nc.sync.dma_start(wsb[:, :NTAPS * COUT // 2],
                                      wb[:, :NTAPS * COUT // 2])
                    nc.scalar.dma_start(wsb[:, NTAPS * COUT // 2:],
                                        wb[:, NTAPS * COUT // 2:])
                    nc.gpsimd.dma_start(scb[:], sc[:])
                    nc.gpsimd.dma_start(shb[:], sh[:])
                # bottom half: x rows 31..55
                xfb = xpool.tile([CIN, B_XROWS * W], f32, tag="xfb",
                                 name=f"xfb{n}")
                xlo = H - B_XROWS
                if n == 0:
                    for qi, (r0, nr) in enumerate(split_rows(xlo, B_XROWS, 3)):
                        dma_engs[qi].dma_start(
                            xfb[:, (r0 - xlo) * W: (r0 - xlo + nr) * W],
                            xs[n][:, r0 * W: (r0 + nr) * W])
                else:
                    nc.scalar.dma_start(xfb[:], xs[n][:, xlo * W: H * W])
                nc.vector.tensor_scalar(xfb[:], xfb[:], 63.75, float(MAGIC),
                                        op0=mult, op1=add)
                nc.scalar.activation(
                    b3[:, 0:B_XROWS, 1:1 + W],
                    xfb.rearrange("p (h w) -> p h w", w=W),
                    mybir.ActivationFunctionType.Copy,
                    bias=float(-MAGIC),
                )

                # chunk-outer so each chunk's epilogue + store overlap the
                # next chunk's matmuls (LDWEIGHTS pipelines in the PE's
                # background weight buffer either way)
                for c in range(COUT_TILES):
                    for r in range(NCHUNKS):
                        psum = pspool.tile(
                            [128, NFREE], f32, tag=f"ps{psum_seq % 8}",
                            name=f"ps_{n}_{c}_{r}")
                        psum_seq += 1
                        for t in range(NTAPS):
                            kh, kw = divmod(t, KK)
                            lw = wsb[:, t * COUT + c * 128:
                                     t * COUT + c * 128 + 128]
                            row0 = r * RPC + kh
                            if r < 4:
                                rhs = a3[:, row0: row0 + RPC, kw: kw + W]
                            else:
                                rhs = b3[:, row0 - B_BASE: row0 - B_BASE + RPC,
                                         kw: kw + W]
                            nc.tensor.matmul(
                                psum[:], lw, rhs,
                                start=(t == 0), stop=(t == NTAPS - 1),
                            )
                        ot = opool.tile([128, NFREE], f32, tag="ot",
                                        name=f"ot_{n}_{c}_{r}")
                        if out_seq % 2 == 0:
                            nc.vector.tensor_scalar(
                                ot[:], psum[:],
                                scb[:, c: c + 1], shb[:, c: c + 1],
                                op0=mult, op1=add,
                            )
                        else:
                            nc.scalar.activation(
                                ot[:], psum[:],
                                mybir.ActivationFunctionType.Identity,
                                bias=shb[:, c: c + 1],
                                scale=scb[:, c: c + 1],
                            )
                        dma_engs[out_seq % 3].dma_start(
                            ys[n, c * 128: (c + 1) * 128,
                               r * NFREE: (r + 1) * NFREE],
                            ot[:],
                        )
                        out_seq += 1

    nc.compile()
    return nc


def _get_nc():
    global _cached_nc
    if _cached_nc is None:
        _cached_nc = _build()
    return _cached_nc


def kernel(x, weight, bias, gamma, beta, running_mean, running_var):
    global LAST_RESULT
    x = np.asarray(x, dtype=np.float32)
    weight = np.asarray(weight, dtype=np.float32)
    bias = np.asarray(bias, dtype=np.float32)
    gamma = np.asarray(gamma, dtype=np.float32)
    beta = np.asarray(beta, dtype=np.float32)
    running_mean = np.asarray(running_mean, dtype=np.float32)
    running_var = np.asarray(running_var, dtype=np.float32)

    # ---- host-side parameter prep (tiny: 295K weights + 256-elem BN math) ----
    # DoReFa weight quantization, f32 ops mirroring the jax reference.
    wt = np.tanh(weight)
    wt = wt / np.abs(wt).max()
    k = np.round(wt * np.float32(127.5) + np.float32(127.5)).astype(np.float32)
    b_int = np.float32(2.0) * k - np.float32(255.0)  # odd ints in [-255, 255]
    # [Cout, Cin, kh, kw] -> [Cin, (tap, Cout)], exact in bf16
    wb_host = np.ascontiguousarray(
        b_int.transpose(1, 2, 3, 0).reshape(CIN, NTAPS * COUT)
    ).astype(ml_dtypes.bfloat16)

    inv = gamma * (np.float32(1.0) / np.sqrt(running_var + np.float32(1e-5)))
    shift = beta - running_mean * inv + bias
    # conv(xq, wq) = (4 / 255^2) * conv(a, b)
    scale = inv * np.float32(4.0 / 65025.0)
    sc_host = np.ascontiguousarray(scale.reshape(COUT_TILES, 128).T)
    sh_host = np.ascontiguousarray(shift.reshape(COUT_TILES, 128).T)

    nc = _get_nc()
    in_maps = []
    for core in range(N_CORES):
        xs_c = np.ascontiguousarray(
            x[core * IMGS:(core + 1) * IMGS].reshape(IMGS, CIN, HW)
        )
        in_maps.append({"xs": xs_c, "wb": wb_host, "sc": sc_host, "sh": sh_host})

    res = run_bass_kernel_spmd(nc, in_maps, list(range(N_CORES)), trace=TRACE,
                               tmpdir=TRACE_DIR)
    LAST_RESULT = res

    out = np.empty((N_BATCH, COUT, H, W), dtype=np.float32)
    for core in range(N_CORES):
        out[core * IMGS:(core + 1) * IMGS] = (
            res.results[core]["ys"].reshape(IMGS, COUT, H, W)
        )
    return out


# revision 13
# speedup vs baseline: 1.0743x; 1.0743x over previous
"""Fused quantized Conv2D + BatchNorm block for Trainium2 (8 NeuronCores).

Reference computation (shapes hardcoded):
  x:      [32, 128, 56, 56] f32    activations in [0, 1)
  weight: [256, 128, 3, 3]  f32
  bias/gamma/beta/running_mean/running_var: [256] f32

  xq = round(clip(x,0,4) * 255/4) * (4/255)          (8-bit act quant)
  wq = DoReFa 8-bit weight quant -> values (2k-255)/255, k in 0..255
  y  = conv2d(xq, wq, stride 1, pad 1)               NCHW/OIHW
  out = y * inv + shift      inv = gamma*rsqrt(var+eps), shift = beta - mean*inv + bias

Kernel strategy:
  - Data-parallel over batch: core i handles images [4i, 4i+4).
  - Integer form: a = round(x*63.75) in {0..255}, b = wq*255 odd ints in
    [-255, 255].  Both are exactly representable in bf16, so the conv is
    done as bf16 matmuls with exact fp32 PSUM accumulation; the final
    per-channel scale folds the 4/255^2 factor and BN into one DVE FMA.
  - Conv as implicit GEMM: Cin=128 on the partition (contraction) dim,
    9 tap matmuls accumulate into PSUM.  The quantized input lives
    zero-padded in SBUF (58-wide rows) so each tap is a strided
    [128, 8, 56] view.  Output chunk = 8 rows * 56 cols = 448 <= 512
    (one PSUM bank); the 8 PSUM banks are used round-robin.
  - Each image is split into top (output rows 0..31) / bottom (32..55)
    halves with a 2-row halo so matmuls start after a half-image
    DMA+quantize instead of a full one.
"""

import numpy as np
import ml_dtypes

import concourse.bacc as bacc
import concourse.tile as tile
from concourse import mybir
from concourse.bass_utils import run_bass_kernel_spmd

N_CORES = 8
N_BATCH = 32
IMGS = N_BATCH // N_CORES  # images per core
CIN = 128
COUT = 256
H = W = 56
HW = H * W
WP = 58  # padded row width
KK = 3
NTAPS = KK * KK
RPC = 8  # output rows per chunk
NCHUNKS = H // RPC  # 7
NFREE = RPC * W  # 448
COUT_TILES = COUT // 128  # 2

# top half: padded rows 0..33 (pad row + x rows 0..32), serves chunks 0..3
A_ROWS = 34
A_XROWS = 33  # x rows 0..32
# bottom half: padded rows 32..57 (x rows 31..55 + pad row), serves chunks 4..6
B_ROWS = 26
B_XROWS = 25  # x rows 31..55
B_BASE = 32  # global padded row of local row 0

MAGIC = np.float32(2.0**23)

TRACE = False
TRACE_DIR = None
LAST_RESULT = None  # BassKernelResults of the most recent run (for profiling)

_cached_nc = None


def _build():
    f32 = mybir.dt.float32
    bf16 = mybir.dt.bfloat16
    mult = mybir.AluOpType.mult
    add = mybir.AluOpType.add

    nc = bacc.Bacc("TRN2", target_bir_lowering=False, debug=False,
                   num_devices=N_CORES)
    xs = nc.dram_tensor("xs", [IMGS, CIN, HW], f32, kind="ExternalInput").ap()
    wb = nc.dram_tensor("wb", [CIN, NTAPS * COUT], bf16, kind="ExternalInput").ap()
    sc = nc.dram_tensor("sc", [128, COUT_TILES], f32, kind="ExternalInput").ap()
    sh = nc.dram_tensor("sh", [128, COUT_TILES], f32, kind="ExternalInput").ap()
    ys = nc.dram_tensor("ys", [IMGS, COUT, HW], f32, kind="ExternalOutput").ap()

    with tile.TileContext(nc) as tc:
        with (
            tc.tile_pool(name="wpool", bufs=1) as wpool,
            tc.tile_pool(name="ppool", bufs=1) as ppool,
            tc.tile_pool(name="xpool", bufs=2) as xpool,
            tc.tile_pool(name="apool", bufs=1) as apool,
            tc.tile_pool(name="opool", bufs=16) as opool,
            tc.tile_pool(name="pspool", bufs=1, space="PSUM") as pspool,
        ):
            # Tiny ACT op up front so the activation-table load (~1.3us)
            # overlaps the first input DMA instead of sitting on the
            # quantize critical path.
            warm = ppool.tile([128, 1], f32, tag="warm")
            nc.vector.memset(warm[:], 0.0)
            nc.scalar.activation(warm[:], warm[:],
                                 mybir.ActivationFunctionType.Copy, bias=0.0)

            # Persistent zero-padded activation buffers (2 images in flight).
            # Pad borders are zeroed once; interiors are rewritten per image.
            # Image-0's buffers are zeroed first; image-1's are deferred so
            # they don't delay gpsimd's other preamble work.
            apadsA, apadsB = [None, None], [None, None]
            araw, braw = [None, None], [None, None]
            for i in range(2):
                ta = apool.tile([CIN, A_ROWS * WP], bf16, tag=f"apadA{i}",
                                name=f"apadA{i}")
                apadsA[i] = ta.rearrange("p (h w) -> p h w", w=WP)
                araw[i] = ta
                tb = apool.tile([CIN, B_ROWS * WP], bf16, tag=f"apadB{i}",
                                name=f"apadB{i}")
                apadsB[i] = tb.rearrange("p (h w) -> p h w", w=WP)
                braw[i] = tb
            nc.gpsimd.memset(araw[0][:], 0.0)
            nc.gpsimd.memset(braw[0][:], 0.0)

            wsb = wpool.tile([CIN, NTAPS * COUT], bf16)
            scb = ppool.tile([128, COUT_TILES], f32, tag="scb")
            shb = ppool.tile([128, COUT_TILES], f32, tag="shb")

            # Per-queue DMA bandwidth caps at ~90 GB/s; only sync/scalar
            # (HWDGE) and gpsimd (SWDGE) can initiate DMAs, so spread
            # traffic across all three queues.
            dma_engs = [nc.sync, nc.scalar, nc.gpsimd]

            def split_rows(lo_row, n_rows, parts):
                base = n_rows // parts
                rows = [base + (1 if i < n_rows % parts else 0)
                        for i in range(parts)]
                out, acc = [], lo_row
                for r in rows:
                    out.append((acc, r))
                    acc += r
                return out

            psum_seq = 0
            out_seq = 0
            for n in range(IMGS):
                a3, b3 = apadsA[n % 2], apadsB[n % 2]
                # top half: x rows 0..32
                xfa = xpool.tile([CIN, A_XROWS * W], f32, tag="xfa",
                                 name=f"xfa{n}")
                if n == 0:
                    # image 0 is on the critical path: split the load across
                    # both fast HWDGE queues (gpsimd/SWDGE starts too slowly)
                    for qi, (r0, nr) in enumerate(split_rows(0, A_XROWS, 2)):
                        dma_engs[qi].dma_start(
                            xfa[:, r0 * W: (r0 + nr) * W],
                            xs[n][:, r0 * W: (r0 + nr) * W])
                else:
                    nc.sync.dma_start(xfa[:], xs[n][:, 0:A_XROWS * W])
                nc.vector.tensor_scalar(xfa[:], xfa[:], 63.75, float(MAGIC),
                                        op0=mult, op1=add)
                nc.scalar.activation(
                    a3[:, 1:1 + A_XROWS, 1:1 + W],
                    xfa.rearrange("p (h w) -> p h w", w=W),
                    mybir.ActivationFunctionType.Copy,
                    bias=float(-MAGIC),
                )
                if n == 0:
                    # weights next on the two fast queues (needed by first MM)
                    nc.sync.dma_start(wsb[:, :NTAPS * COUT // 2],
                                      wb[:, :NTAPS * COUT // 2])
                    nc.scalar.dma_start(wsb[:, NTAPS * COUT // 2:],
                                        wb[:, NTAPS * COUT // 2:])
                    nc.gpsimd.dma_start(scb[:], sc[:])
                    nc.gpsimd.dma_start(shb[:], sh[:])
                # bottom half: x rows 31..55
                xfb = xpool.tile([CIN, B_XROWS * W], f32, tag="xfb",
                                 name=f"xfb{n}")
                xlo = H - B_XROWS
                if n == 0:
                    for qi, (r0, nr) in enumerate(split_rows(xlo, B_XROWS, 2)):
                        dma_engs[qi].dma_start(
                            xfb[:, (r0 - xlo) * W: (r0 - xlo + nr) * W],
                            xs[n][:, r0 * W: (r0 + nr) * W])
                    # deferred: image-1 buffer zeroing + the next prefetches
                    nc.gpsimd.memset(araw[1][:], 0.0)
                    nc.gpsimd.memset(braw[1][:], 0.0)
                else:
                    nc.scalar.dma_start(xfb[:], xs[n][:, xlo * W: H * W])
                nc.vector.tensor_scalar(xfb[:], xfb[:], 63.75, float(MAGIC),
                                        op0=mult, op1=add)
                nc.scalar.activation(
                    b3[:, 0:B_XROWS, 1:1 + W],
                    xfb.rearrange("p (h w) -> p h w", w=W),
                    mybir.ActivationFunctionType.Copy,
                    bias=float(-MAGIC),
                )

                # chunk-outer so each chunk's epilogue + store overlap the
                # next chunk's matmuls (LDWEIGHTS pipelines in the PE's
                # background weight buffer either way)
                for c in range(COUT_TILES):
                    for r in range(NCHUNKS):
                        psum = pspool.tile(
                            [128, NFREE], f32, tag=f"ps{psum_seq % 8}",
                            name=f"ps_{n}_{c}_{r}")
                        psum_seq += 1
                        for t in range(NTAPS):
                            kh, kw = divmod(t, KK)
                            lw = wsb[:, t * COUT + c * 128:
                                     t * COUT + c * 128 + 128]
                            row0 = r * RPC + kh
                            if r < 4:
                                rhs = a3[:, row0: row0 + RPC, kw: kw + W]
                            else:
                                rhs = b3[:, row0 - B_BASE: row0 - B_BASE + RPC,
                                         kw: kw + W]
                            nc.tensor.matmul(
                                psum[:], lw, rhs,
                                start=(t == 0), stop=(t == NTAPS - 1),
                            )
                        ot = opool.tile([128, NFREE], f32, tag="ot",
                                        name=f"ot_{n}_{c}_{r}")
                        if out_seq % 2 == 0:
                            nc.vector.tensor_scalar(
                                ot[:], psum[:],
                                scb[:, c: c + 1], shb[:, c: c + 1],
                                op0=mult, op1=add,
                            )
                        else:
                            nc.scalar.activation(
                                ot[:], psum[:],
                                mybir.ActivationFunctionType.Identity,
                                bias=shb[:, c: c + 1],
                                scale=scb[:, c: c + 1],
                            )
                        dma_engs[out_seq % 3].dma_start(
                            ys[n, c * 128: (c + 1) * 128,
                               r * NFREE: (r + 1) * NFREE],
                            ot[:],
                        )
                        out_seq += 1

    nc.compile()
    return nc


def _get_nc():
    global _cached_nc
    if _cached_nc is None:
        _cached_nc = _build()
    return _cached_nc


def kernel(x, weight, bias, gamma, beta, running_mean, running_var):
    global LAST_RESULT
    x = np.asarray(x, dtype=np.float32)
    weight = np.asarray(weight, dtype=np.float32)
    bias = np.asarray(bias, dtype=np.float32)
    gamma = np.asarray(gamma, dtype=np.float32)
    beta = np.asarray(beta, dtype=np.float32)
    running_mean = np.asarray(running_mean, dtype=np.float32)
    running_var = np.asarray(running_var, dtype=np.float32)

    # ---- host-side parameter prep (tiny: 295K weights + 256-elem BN math) ----
    # DoReFa weight quantization, f32 ops mirroring the jax reference.
    wt = np.tanh(weight)
    wt = wt / np.abs(wt).max()
    k = np.round(wt * np.float32(127.5) + np.float32(127.5)).astype(np.float32)
    b_int = np.float32(2.0) * k - np.float32(255.0)  # odd ints in [-255, 255]
    # [Cout, Cin, kh, kw] -> [Cin, (tap, Cout)], exact in bf16
    wb_host = np.ascontiguousarray(
        b_int.transpose(1, 2, 3, 0).reshape(CIN, NTAPS * COUT)
    ).astype(ml_dtypes.bfloat16)

    inv = gamma * (np.float32(1.0) / np.sqrt(running_var + np.float32(1e-5)))
    shift = beta - running_mean * inv + bias
    # conv(xq, wq) = (4 / 255^2) * conv(a, b)
    scale = inv * np.float32(4.0 / 65025.0)
    sc_host = np.ascontiguousarray(scale.reshape(COUT_TILES, 128).T)
    sh_host = np.ascontiguousarray(shift.reshape(COUT_TILES, 128).T)

    nc = _get_nc()
    in_maps = []
    for core in range(N_CORES):
        xs_c = np.ascontiguousarray(
            x[core * IMGS:(core + 1) * IMGS].reshape(IMGS, CIN, HW)
        )
        in_maps.append({"xs": xs_c, "wb": wb_host, "sc": sc_host, "sh": sh_host})

    res = run_bass_kernel_spmd(nc, in_maps, list(range(N_CORES)), trace=TRACE,
                               tmpdir=TRACE_DIR)
    LAST_RESULT = res

    out = np.empty((N_BATCH, COUT, H, W), dtype=np.float32)
    for core in range(N_CORES):
        out[core * IMGS:(core + 1) * IMGS] = (
            res.results[core]["ys"].reshape(IMGS, COUT, H, W)
        )
    return out


# revision 17
# speedup vs baseline: 1.0935x; 1.0179x over previous
"""Fused quantized Conv2D + BatchNorm block for Trainium2 (8 NeuronCores).

Reference computation (shapes hardcoded):
  x:      [32, 128, 56, 56] f32    activations in [0, 1)
  weight: [256, 128, 3, 3]  f32
  bias/gamma/beta/running_mean/running_var: [256] f32

  xq = round(clip(x,0,4) * 255/4) * (4/255)          (8-bit act quant)
  wq = DoReFa 8-bit weight quant -> values (2k-255)/255, k in 0..255
  y  = conv2d(xq, wq, stride 1, pad 1)               NCHW/OIHW
  out = y * inv + shift      inv = gamma*rsqrt(var+eps), shift = beta - mean*inv + bias

Kernel strategy:
  - Data-parallel over batch: core i handles images [4i, 4i+4).
  - Integer form: a = round(x*63.75) in {0..255}, b = wq*255 odd ints in
    [-255, 255].  Both are exactly representable in bf16, so the conv is
    done as bf16 matmuls with exact fp32 PSUM accumulation; the final
    per-channel scale folds the 4/255^2 factor and BN into one DVE FMA.
  - Conv as implicit GEMM: Cin=128 on the partition (contraction) dim,
    9 tap matmuls accumulate into PSUM.  The quantized input lives
    zero-padded in SBUF (58-wide rows) so each tap is a strided
    [128, 8, 56] view.  Output chunk = 8 rows * 56 cols = 448 <= 512
    (one PSUM bank); the 8 PSUM banks are used round-robin.
  - Each image is split into top (output rows 0..31) / bottom (32..55)
    halves with a 2-row halo so matmuls start after a half-image
    DMA+quantize instead of a full one.
"""

import numpy as np
import ml_dtypes

import concourse.bacc as bacc
import concourse.tile as tile
from concourse import mybir
from concourse.bass_utils import run_bass_kernel_spmd

N_CORES = 8
N_BATCH = 32
IMGS = N_BATCH // N_CORES  # images per core
CIN = 128
COUT = 256
H = W = 56
HW = H * W
WP = 58  # padded row width
KK = 3
NTAPS = KK * KK
RPC = 8  # output rows per chunk
NCHUNKS = H // RPC  # 7
NFREE = RPC * W  # 448
COUT_TILES = COUT // 128  # 2

# top half: padded rows 0..33 (pad row + x rows 0..32), serves chunks 0..3
A_ROWS = 34
A_XROWS = 33  # x rows 0..32
# bottom half: padded rows 32..57 (x rows 31..55 + pad row), serves chunks 4..6
B_ROWS = 26
B_XROWS = 25  # x rows 31..55
B_BASE = 32  # global padded row of local row 0

MAGIC = np.float32(2.0**23)

TRACE = False
TRACE_DIR = None
LAST_RESULT = None  # BassKernelResults of the most recent run (for profiling)

_cached_nc = None


def _build():
    f32 = mybir.dt.float32
    bf16 = mybir.dt.bfloat16
    mult = mybir.AluOpType.mult
    add = mybir.AluOpType.add

    nc = bacc.Bacc("TRN2", target_bir_lowering=False, debug=False,
                   num_devices=N_CORES)
    xs = nc.dram_tensor("xs", [IMGS, CIN, HW], f32, kind="ExternalInput").ap()
    wb = nc.dram_tensor("wb", [CIN, NTAPS * COUT], bf16, kind="ExternalInput").ap()
    sc = nc.dram_tensor("sc", [128, COUT_TILES], f32, kind="ExternalInput").ap()
    sh = nc.dram_tensor("sh", [128, COUT_TILES], f32, kind="ExternalInput").ap()
    ys = nc.dram_tensor("ys", [IMGS, COUT, HW], f32, kind="ExternalOutput").ap()

    with tile.TileContext(nc) as tc:
        with (
            tc.tile_pool(name="wpool", bufs=1) as wpool,
            tc.tile_pool(name="ppool", bufs=1) as ppool,
            tc.tile_pool(name="xpool", bufs=2) as xpool,
            tc.tile_pool(name="apool", bufs=1) as apool,
            tc.tile_pool(name="opool", bufs=16) as opool,
            tc.tile_pool(name="pspool", bufs=1, space="PSUM") as pspool,
        ):
            # Tiny ACT op up front so the activation-table load (~1.3us)
            # overlaps the first input DMA instead of sitting on the
            # quantize critical path.
            warm = ppool.tile([128, 1], f32, tag="warm")
            nc.vector.memset(warm[:], 0.0)
            nc.scalar.activation(warm[:], warm[:],
                                 mybir.ActivationFunctionType.Copy, bias=0.0)

            # Dummy matmuls keep the PE busy through the input-load preamble
            # so the HAM clock gate is already at 8/8 (2.4 GHz) when the real
            # stream starts (it would otherwise run ~3.4us at 1.2 GHz).
            dummy = wpool.tile([128, 640], bf16, tag="dummy")
            nc.vector.memset(dummy[:], 0.0)
            dps = pspool.tile([128, 512], f32, tag="psd", name="ps_dummy")
            for i in range(30):
                nc.tensor.matmul(dps[:], dummy[:, :128], dummy[:, 128:640],
                                 start=True, stop=True)

            # Persistent zero-padded activation buffers (2 images in flight).
            # Pad borders are zeroed once; interiors are rewritten per image.
            # Image-0's buffers are zeroed first; image-1's are deferred so
            # they don't delay gpsimd's other preamble work.
            apadsA, apadsB = [None, None], [None, None]
            araw, braw = [None, None], [None, None]
            for i in range(2):
                ta = apool.tile([CIN, A_ROWS * WP], bf16, tag=f"apadA{i}",
                                name=f"apadA{i}")
                apadsA[i] = ta.rearrange("p (h w) -> p h w", w=WP)
                araw[i] = ta
                tb = apool.tile([CIN, B_ROWS * WP], bf16, tag=f"apadB{i}",
                                name=f"apadB{i}")
                apadsB[i] = tb.rearrange("p (h w) -> p h w", w=WP)
                braw[i] = tb
            nc.gpsimd.memset(araw[0][:], 0.0)
            nc.gpsimd.memset(braw[0][:], 0.0)

            wsb = wpool.tile([CIN, NTAPS * COUT], bf16)
            scb = ppool.tile([128, COUT_TILES], f32, tag="scb")
            shb = ppool.tile([128, COUT_TILES], f32, tag="shb")

            # Per-queue DMA bandwidth caps at ~90 GB/s; only sync/scalar
            # (HWDGE) and gpsimd (SWDGE) can initiate DMAs, so spread
            # traffic across all three queues.
            dma_engs = [nc.sync, nc.scalar, nc.gpsimd]

            def split_rows(lo_row, n_rows, parts):
                base = n_rows // parts
                rows = [base + (1 if i < n_rows % parts else 0)
                        for i in range(parts)]
                out, acc = [], lo_row
                for r in rows:
                    out.append((acc, r))
                    acc += r
                return out

            psum_seq = 0
            out_seq = 0
            for n in range(IMGS):
                a3, b3 = apadsA[n % 2], apadsB[n % 2]
                # top half: x rows 0..32
                xfa = xpool.tile([CIN, A_XROWS * W], f32, tag="xfa",
                                 name=f"xfa{n}")
                if n == 0:
                    # image 0 is on the critical path: split the load across
                    # both fast HWDGE queues (gpsimd/SWDGE starts too slowly)
                    for qi, (r0, nr) in enumerate(split_rows(0, A_XROWS, 2)):
                        dma_engs[qi].dma_start(
                            xfa[:, r0 * W: (r0 + nr) * W],
                            xs[n][:, r0 * W: (r0 + nr) * W])
                else:
                    nc.sync.dma_start(xfa[:], xs[n][:, 0:A_XROWS * W])
                xfa3 = xfa.rearrange("p (h w) -> p h w", w=W)
                if n == 0:
                    # split quantize across engines to shorten the critical
                    # path to the first matmul
                    lo = 17
                    nc.vector.tensor_scalar(xfa[:, :lo * W], xfa[:, :lo * W],
                                            63.75, float(MAGIC),
                                            op0=mult, op1=add)
                    nc.gpsimd.tensor_scalar(xfa[:, lo * W:], xfa[:, lo * W:],
                                            63.75, float(MAGIC),
                                            op0=mult, op1=add)
                    nc.scalar.activation(
                        a3[:, 1:1 + lo, 1:1 + W],
                        xfa3[:, 0:lo, :],
                        mybir.ActivationFunctionType.Copy,
                        bias=float(-MAGIC),
                    )
                    nc.vector.tensor_scalar_add(
                        a3[:, 1 + lo:1 + A_XROWS, 1:1 + W],
                        xfa3[:, lo:A_XROWS, :],
                        float(-MAGIC),
                    )
                else:
                    nc.vector.tensor_scalar(xfa[:], xfa[:], 63.75,
                                            float(MAGIC), op0=mult, op1=add)
                    nc.scalar.activation(
                        a3[:, 1:1 + A_XROWS, 1:1 + W],
                        xfa3[:],
                        mybir.ActivationFunctionType.Copy,
                        bias=float(-MAGIC),
                    )
                if n == 0:
                    # weights next on the two fast queues (needed by first MM)
                    nc.sync.dma_start(wsb[:, :NTAPS * COUT // 2],
                                      wb[:, :NTAPS * COUT // 2])
                    nc.scalar.dma_start(wsb[:, NTAPS * COUT // 2:],
                                        wb[:, NTAPS * COUT // 2:])
                    nc.gpsimd.dma_start(scb[:], sc[:])
                    nc.gpsimd.dma_start(shb[:], sh[:])
                # bottom half: x rows 31..55
                xfb = xpool.tile([CIN, B_XROWS * W], f32, tag="xfb",
                                 name=f"xfb{n}")
                xlo = H - B_XROWS
                if n == 0:
                    for qi, (r0, nr) in enumerate(split_rows(xlo, B_XROWS, 2)):
                        dma_engs[qi].dma_start(
                            xfb[:, (r0 - xlo) * W: (r0 - xlo + nr) * W],
                            xs[n][:, r0 * W: (r0 + nr) * W])
                    # deferred: image-1 buffer zeroing + the next prefetches
                    nc.gpsimd.memset(araw[1][:], 0.0)
                    nc.gpsimd.memset(braw[1][:], 0.0)
                else:
                    nc.scalar.dma_start(xfb[:], xs[n][:, xlo * W: H * W])
                nc.vector.tensor_scalar(xfb[:], xfb[:], 63.75, float(MAGIC),
                                        op0=mult, op1=add)
                nc.scalar.activation(
                    b3[:, 0:B_XROWS, 1:1 + W],
                    xfb.rearrange("p (h w) -> p h w", w=W),
                    mybir.ActivationFunctionType.Copy,
                    bias=float(-MAGIC),
                )

                # chunk-outer so each chunk's epilogue + store overlap the
                # next chunk's matmuls (LDWEIGHTS pipelines in the PE's
                # background weight buffer either way)
                for c in range(COUT_TILES):
                    for r in range(NCHUNKS):
                        psum = pspool.tile(
                            [128, NFREE], f32, tag=f"ps{psum_seq % 7}",
                            name=f"ps_{n}_{c}_{r}")
                        psum_seq += 1
                        for t in range(NTAPS):
                            kh, kw = divmod(t, KK)
                            lw = wsb[:, t * COUT + c * 128:
                                     t * COUT + c * 128 + 128]
                            row0 = r * RPC + kh
                            if r < 4:
                                rhs = a3[:, row0: row0 + RPC, kw: kw + W]
                            else:
                                rhs = b3[:, row0 - B_BASE: row0 - B_BASE + RPC,
                                         kw: kw + W]
                            nc.tensor.matmul(
                                psum[:], lw, rhs,
                                start=(t == 0), stop=(t == NTAPS - 1),
                            )
                        ot = opool.tile([128, NFREE], f32, tag="ot",
                                        name=f"ot_{n}_{c}_{r}")
                        if out_seq % 2 == 0:
                            nc.vector.tensor_scalar(
                                ot[:], psum[:],
                                scb[:, c: c + 1], shb[:, c: c + 1],
                                op0=mult, op1=add,
                            )
                        else:
                            nc.scalar.activation(
                                ot[:], psum[:],
                                mybir.ActivationFunctionType.Identity,
                                bias=shb[:, c: c + 1],
                                scale=scb[:, c: c + 1],
                            )
                        dma_engs[out_seq % 3].dma_start(
                            ys[n, c * 128: (c + 1) * 128,
                               r * NFREE: (r + 1) * NFREE],
                            ot[:],
                        )
                        out_seq += 1

    nc.compile()
    return nc


def _get_nc():
    global _cached_nc
    if _cached_nc is None:
        _cached_nc = _build()
    return _cached_nc


def kernel(x, weight, bias, gamma, beta, running_mean, running_var):
    global LAST_RESULT
    x = np.asarray(x, dtype=np.float32)
    weight = np.asarray(weight, dtype=np.float32)
    bias = np.asarray(bias, dtype=np.float32)
    gamma = np.asarray(gamma, dtype=np.float32)
    beta = np.asarray(beta, dtype=np.float32)
    running_mean = np.asarray(running_mean, dtype=np.float32)
    running_var = np.asarray(running_var, dtype=np.float32)

    # ---- host-side parameter prep (tiny: 295K weights + 256-elem BN math) ----
    # DoReFa weight quantization, f32 ops mirroring the jax reference.
    wt = np.tanh(weight)
    wt = wt / np.abs(wt).max()
    k = np.round(wt * np.float32(127.5) + np.float32(127.5)).astype(np.float32)
    b_int = np.float32(2.0) * k - np.float32(255.0)  # odd ints in [-255, 255]
    # [Cout, Cin, kh, kw] -> [Cin, (tap, Cout)], exact in bf16
    wb_host = np.ascontiguousarray(
        b_int.transpose(1, 2, 3, 0).reshape(CIN, NTAPS * COUT)
    ).astype(ml_dtypes.bfloat16)

    inv = gamma * (np.float32(1.0) / np.sqrt(running_var + np.float32(1e-5)))
    shift = beta - running_mean * inv + bias
    # conv(xq, wq) = (4 / 255^2) * conv(a, b)
    scale = inv * np.float32(4.0 / 65025.0)
    sc_host = np.ascontiguousarray(scale.reshape(COUT_TILES, 128).T)
    sh_host = np.ascontiguousarray(shift.reshape(COUT_TILES, 128).T)

    nc = _get_nc()
    in_maps = []
    for core in range(N_CORES):
        xs_c = np.ascontiguousarray(
            x[core * IMGS:(core + 1) * IMGS].reshape(IMGS, CIN, HW)
        )
        in_maps.append({"xs": xs_c, "wb": wb_host, "sc": sc_host, "sh": sh_host})

    res = run_bass_kernel_spmd(nc, in_maps, list(range(N_CORES)), trace=TRACE,
                               tmpdir=TRACE_DIR)
    LAST_RESULT = res

    out = np.empty((N_BATCH, COUT, H, W), dtype=np.float32)
    for core in range(N_CORES):
        out[core * IMGS:(core + 1) * IMGS] = (
            res.results[core]["ys"].reshape(IMGS, COUT, H, W)
        )
    return out
